# revision 17
# baseline (speedup 1.0000x reference)
"""Trainium2 Bass kernel for Mobile2Former cross-attention block.

Computation (per batch b):
    xf   = x[b].reshape(C, H*W)                      # [64, 3136] keys=values
    q    = (z[b] @ Wq + bq).reshape(heads, M, C)     # [8, 6, 64]
    attn = softmax(q @ xf * C**-0.5, axis=-1)        # [8, 6, 3136]
    res  = attn @ xf.T                               # [8, 6, 64]
    out  = res.transpose(1,0,2).reshape(M, -1) @ Wo + bo + z[b]

Strategy: data-parallel over B across 8 cores (16 batches/core), batches
processed in pairs (two batches stacked on the 128 SBUF partitions, C=64
each).  QK^T is computed directly in transposed layout (attn^T[n, hm]) by
using xf chunks as the matmul stationary operand; the same stationary is
reused by an identity matmul to produce xf^T chunks for the AV matmul.
Softmax runs without max subtraction (logits are O(1)); the denominator
comes for free from a ones-column appended to the AV moving operand.
x and Wo are pre-cast to bf16 on host; the attention scale is folded into
Wq/bq on host.
"""

import sys
from contextlib import ExitStack

import numpy as np

sys.path.insert(0, "/opt/trn_rl_repo")

import concourse.bass as bass
import concourse.tile as tile
from concourse import bacc as bacc_mod
from concourse import mybir
from concourse.bass_utils import run_bass_kernel_spmd

try:
    import ml_dtypes

    BF16 = ml_dtypes.bfloat16
except ImportError:  # pragma: no cover
    import jax.numpy as jnp

    BF16 = jnp.bfloat16

N_CORES = 8
B, C, H, W = 128, 64, 56, 56
HW = H * W  # 3136
M, D = 6, 192
NH = 8
INNER = NH * C  # 512
BPC = B // N_CORES  # 16 batches per core
NPAIR = BPC // 2  # 8 pairs per core
NCHUNK = (HW + 127) // 128  # 25 (24 full + one 64-wide)

F32 = mybir.dt.float32
BF = mybir.dt.bfloat16

_CACHE = {}


def _build_nc() -> bass.Bass:
    nc = bacc_mod.Bacc()

    x_h = nc.declare_dram_parameter("x", [BPC, C, H, W], BF, isOutput=False)
    z_h = nc.declare_dram_parameter("z", [BPC, M, D], F32, isOutput=False)
    zbo_h = nc.declare_dram_parameter("zbo", [BPC, M, D], F32, isOutput=False)
    wq_h = nc.declare_dram_parameter("wq", [D, INNER], F32, isOutput=False)
    bqt_h = nc.declare_dram_parameter("bqt", [128, 4], F32, isOutput=False)
    wo_h = nc.declare_dram_parameter("wo", [INNER, D], BF, isOutput=False)
    idb_h = nc.declare_dram_parameter("ident_bf", [128, 128], BF, isOutput=False)
    idf_h = nc.declare_dram_parameter("ident_f32", [128, 128], F32, isOutput=False)
    out_h = nc.declare_dram_parameter("out", [BPC, M, D], F32, isOutput=True)

    # DRAM views
    x_r = x_h.ap().rearrange("b c h w -> (b c) (h w)")  # [1024, 3136]
    z_r = z_h.ap().rearrange("b m d -> (b m) d")  # [96, 192]
    # [12(t,m), 8(pair), 192(d)]: partition q=6t+m, free (pair, d)
    zbo_r = bass.AP(
        tensor=zbo_h.ap().tensor, offset=0,
        ap=[[D, 2 * M], [2 * M * D, NPAIR], [1, D]],
    )
    out_r = bass.AP(
        tensor=out_h.ap().tensor, offset=0,
        ap=[[D, 2 * M], [2 * M * D, NPAIR], [1, D]],
    )
    # [128(p), 4(k), 192(d)]: wo[k*128+p, d]
    wo_r = bass.AP(
        tensor=wo_h.ap().tensor, offset=0,
        ap=[[D, 128], [128 * D, 4], [1, D]],
    )

    with tile.TileContext(nc) as tc, ExitStack() as ctx:
        const = ctx.enter_context(tc.tile_pool(name="const", bufs=1))
        xf_pool = ctx.enter_context(tc.tile_pool(name="xf", bufs=3))
        ax_pool = ctx.enter_context(tc.tile_pool(name="ax", bufs=3))
        xts_pool = ctx.enter_context(tc.tile_pool(name="xts", bufs=3))
        small = ctx.enter_context(tc.tile_pool(name="small", bufs=3))
        at_psum = ctx.enter_context(tc.tile_pool(name="at_ps", bufs=2, space="PSUM"))
        xt_psum = ctx.enter_context(tc.tile_pool(name="xt_ps", bufs=2, space="PSUM"))
        rs_psum = ctx.enter_context(tc.tile_pool(name="rs_ps", bufs=2, space="PSUM"))
        sm_psum = ctx.enter_context(tc.tile_pool(name="sm_ps", bufs=2, space="PSUM"))

        # ---------------- phase 0: constants / projections ----------------
        ident_bf = const.tile([128, 128], BF)
        nc.sync.dma_start(out=ident_bf, in_=idb_h.ap())
        ident_f32 = const.tile([128, 128], F32)
        nc.sync.dma_start(out=ident_f32, in_=idf_h.ap())

        z_nat = const.tile([96, D], F32)
        nc.sync.dma_start(out=z_nat, in_=z_r)
        zbo_sb = const.tile([12, NPAIR * D], F32)
        nc.sync.dma_start(
            out=zbo_sb.rearrange("q (p d) -> q p d", p=NPAIR), in_=zbo_r
        )
        wq0 = const.tile([128, INNER], F32)
        nc.sync.dma_start(out=wq0, in_=wq_h.ap()[0:128, :])
        wq1 = const.tile([64, INNER], F32)
        nc.sync.dma_start(out=wq1, in_=wq_h.ap()[128:192, :])
        bqt_sb = const.tile([128, 4], F32)
        nc.sync.dma_start(out=bqt_sb, in_=bqt_h.ap())
        wo_sb = const.tile([128, 4 * D], BF)
        nc.sync.dma_start(out=wo_sb.rearrange("q (k d) -> q k d", k=4), in_=wo_r)

        # z^T via PE transpose: [96, 192] -> [192, 96] in two chunks
        zt0p = at_psum.tile([128, 96], F32, tag="at")
        nc.tensor.transpose(zt0p, z_nat[:, 0:128], ident_f32[0:96, 0:96])
        zt1p = at_psum.tile([64, 96], F32, tag="at")
        nc.tensor.transpose(zt1p, z_nat[:, 128:192], ident_f32[0:96, 0:96])
        zt0 = const.tile([128, 96], F32)
        nc.vector.tensor_copy(out=zt0, in_=zt0p)
        zt1 = const.tile([64, 96], F32)
        nc.vector.tensor_copy(out=zt1, in_=zt1p)

        # q^T for all 16 local batches: qT_all[i, 6b+m] = (z @ Wq + bq)^T
        # chunk ii holds i in [128*ii, 128*ii+128)
        qT_all = const.tile([128, 4 * 96], BF)
        for ii in range(4):
            qp = at_psum.tile([128, 96], F32, tag="at", name=f"qp{ii}")
            nc.tensor.matmul(
                qp, lhsT=wq0[:, 128 * ii : 128 * ii + 128], rhs=zt0,
                start=True, stop=False,
            )
            nc.tensor.matmul(
                qp, lhsT=wq1[:, 128 * ii : 128 * ii + 128], rhs=zt1,
                start=False, stop=True,
            )
            nc.vector.tensor_scalar_add(
                out=qT_all[:, 96 * ii : 96 * ii + 96], in0=qp,
                scalar1=bqt_sb[:, ii : ii + 1],
            )

        # ---------------- per-pair main loop ----------------
        # column order inside a pair: hm2 = 48*b + u, u = 6*h + m.
        # Reference's q reshape is a FLAT view of [M, H*C], so the query row
        # for (h, m) is q_flat[(6h+m)//8, 64*((6h+m)%8) : +64].  With
        # u = 8*t + 2*ii + g: source chunk ii, partition half g, z-row t.
        WAVES = [(0, 5), (5, 5), (10, 5), (15, 5), (20, 5)]
        qT_all_g = qT_all.rearrange("p (hh x) -> p hh x", hh=4)  # [128, 4, 96]

        for p in range(NPAIR):
            xf = xf_pool.tile([128, HW], BF, tag="xf", name=f"xf{p}")
            nc.sync.dma_start(out=xf, in_=x_r[128 * p : 128 * (p + 1), :])

            # block-diagonal qT2: [c2, hm2]; c2 = 64*b + c
            qT2 = small.tile([128, 96], BF, tag="qT2", name=f"qT2_{p}")
            nc.gpsimd.memset(qT2, 0.0)
            # col = 48*b + 8*t + 2*ii + g  ->  view [q, b, ii, t, g]
            qT2_v = qT2.rearrange("q (b t ii g) -> q b ii t g", b=2, t=6, ii=4)
            for b in range(2):
                for g in range(2):
                    dst = qT2_v[64 * b : 64 * b + 64, b, :, :, g]
                    src = qT_all_g[
                        64 * g : 64 * g + 64, :, 12 * p + 6 * b : 12 * p + 6 * b + 6
                    ]
                    nc.gpsimd.tensor_copy(out=dst, in_=src)

            rsum = rs_psum.tile([96, 129], F32, tag="rs", name=f"rsum{p}")

            for w, (j0, nj) in enumerate(WAVES):
                at = at_psum.tile([128, 5 * 96], F32, tag="at", name=f"at{p}_{w}")
                xt = xt_psum.tile([128, 5 * 128], BF, tag="xt", name=f"xt{p}_{w}")
                for jj in range(nj):
                    j = j0 + jj
                    cw = 64 if j == NCHUNK - 1 else 128
                    lhs = xf[:, 128 * j : 128 * j + cw]
                    nc.tensor.matmul(
                        out=at[0:cw, 96 * jj : 96 * jj + 96], lhsT=lhs, rhs=qT2,
                        start=True, stop=True,
                    )
                    nc.tensor.transpose(
                        xt[0:cw, 128 * jj : 128 * jj + 128], lhs, ident_bf
                    )

                ax = ax_pool.tile([128, 5 * 96], BF, tag="ax", name=f"ax{p}_{w}")
                xts = xts_pool.tile([128, 5 * 132], BF, tag="xts", name=f"xts{p}_{w}")
                xts_g = xts.rearrange("q (j c) -> q j c", j=5)
                xt_g = xt.rearrange("q (j c) -> q j c", j=5)
                last = w == len(WAVES) - 1

                def cp(out, in_, _w=w):
                    if _w % 2 == 0:
                        nc.vector.tensor_copy(out=out, in_=in_)
                    else:
                        nc.scalar.copy(out=out, in_=in_)

                if not last:
                    nc.scalar.activation(
                        out=ax, in_=at, func=mybir.ActivationFunctionType.Exp
                    )
                    cp(xts_g[:, :, 0:128], xt_g)
                    nc.gpsimd.memset(xts_g[:, :, 128:129], 1.0)
                else:
                    nc.scalar.activation(
                        out=ax[:, 0:384], in_=at[:, 0:384],
                        func=mybir.ActivationFunctionType.Exp,
                    )
                    nc.scalar.activation(
                        out=ax[0:64, 384:480], in_=at[0:64, 384:480],
                        func=mybir.ActivationFunctionType.Exp,
                    )
                    cp(xts_g[:, 0:4, 0:128], xt_g[:, 0:4, :])
                    cp(xts_g[0:64, 4, 0:128], xt_g[0:64, 4, :])
                    nc.gpsimd.memset(xts_g[:, 0:4, 128:129], 1.0)
                    nc.gpsimd.memset(xts_g[0:64, 4, 128:129], 1.0)

                for jj in range(nj):
                    j = j0 + jj
                    cw = 64 if j == NCHUNK - 1 else 128
                    nc.tensor.matmul(
                        out=rsum,
                        lhsT=ax[0:cw, 96 * jj : 96 * jj + 96],
                        rhs=xts[0:cw, 132 * jj : 132 * jj + 129],
                        start=(j == 0), stop=(j == NCHUNK - 1),
                    )

            # softmax denominator is rsum[:, 128]; normalize and emit
            inv = small.tile([96, 1], F32, tag="inv", name=f"inv{p}")
            nc.vector.reciprocal(out=inv, in_=rsum[:, 128:129])
            r2n = small.tile([96, 128], BF, tag="r2n", name=f"r2n{p}")
            nc.vector.tensor_scalar_mul(out=r2n, in0=rsum[:, 0:128], scalar1=inv)

            rt = sm_psum.tile([128, 96], BF, tag="sm", name=f"rt{p}")
            nc.tensor.transpose(rt, r2n, ident_bf[0:96, 0:96])
            rt_sb = small.tile([128, 96], BF, tag="rt_sb", name=f"rt_sb{p}")
            nc.vector.tensor_copy(out=rt_sb, in_=rt)

            # fc lhsT: fcl[64*hl + c, 12*kk + 6*b + m]
            #        = rt_sb[64*b + c, 48*b + 12*kk + 6*hl + m]   (h = 2*kk + hl)
            fcl = small.tile([128, 48], BF, tag="fcl", name=f"fcl{p}")
            fcl_g = fcl.rearrange("q (kk x) -> q kk x", kk=4)
            rt_v = rt_sb.rearrange("q (b kk hl m) -> q b kk hl m", b=2, kk=4, hl=2)
            for hl in range(2):
                for b in range(2):
                    dst = fcl_g[64 * hl : 64 * hl + 64, :, 6 * b : 6 * b + 6]
                    src = rt_v[64 * b : 64 * b + 64, b, :, hl, :]
                    nc.gpsimd.tensor_copy(out=dst, in_=src)

            o2 = sm_psum.tile([12, D], F32, tag="sm", name=f"o2_{p}")
            for kk in range(4):
                nc.tensor.matmul(
                    out=o2, lhsT=fcl[:, 12 * kk : 12 * kk + 12],
                    rhs=wo_sb[:, 192 * kk : 192 * kk + 192],
                    start=(kk == 0), stop=(kk == 3),
                )
            if p == 0:
                out_all = const.tile([12, NPAIR * D], F32)
            nc.vector.tensor_add(
                out=out_all[:, D * p : D * (p + 1)], in0=o2,
                in1=zbo_sb[:, D * p : D * (p + 1)],
            )

        nc.sync.dma_start(
            out=out_r, in_=out_all.rearrange("q (p d) -> q p d", p=NPAIR)
        )

    return nc


def get_nc() -> bass.Bass:
    if "nc" not in _CACHE:
        nc = _build_nc()
        # The PJRT exec path serializes nc.m as-is; run Bacc's legalization
        # (wait splitting, register allocation, ...) explicitly.
        nc.finalize()
        _CACHE["nc"] = nc
    return _CACHE["nc"]


def make_in_maps(x, z, Wq, bq, Wo, bo):
    """Host-side prep + sharding into per-core input maps."""
    x = np.asarray(x, dtype=np.float32)
    z = np.asarray(z, dtype=np.float32)
    Wq = np.asarray(Wq, dtype=np.float32)
    bq = np.asarray(bq, dtype=np.float32)
    Wo = np.asarray(Wo, dtype=np.float32)
    bo = np.asarray(bo, dtype=np.float32)

    scale = np.float32(C ** -0.5)
    x_bf = x.astype(BF16)
    wq_s = (Wq * scale).astype(np.float32)
    bqt = (bq * scale).reshape(4, 128).T.copy()  # [128, 4], chunk ii in col ii
    wo_bf = Wo.astype(BF16)
    zbo = (z + bo[None, None, :]).astype(np.float32)
    ident_bf = np.eye(128, dtype=BF16)
    ident_f32 = np.eye(128, dtype=np.float32)

    in_maps = []
    for i in range(N_CORES):
        s = slice(i * BPC, (i + 1) * BPC)
        in_maps.append(
            {
                "x": x_bf[s],
                "z": z[s],
                "zbo": zbo[s],
                "wq": wq_s,
                "bqt": bqt,
                "wo": wo_bf,
                "ident_bf": ident_bf,
                "ident_f32": ident_f32,
            }
        )
    return in_maps


def kernel(**inputs) -> np.ndarray:
    nc = get_nc()
    in_maps = make_in_maps(
        inputs["x"], inputs["z"], inputs["Wq"], inputs["bq"],
        inputs["Wo"], inputs["bo"],
    )
    res = run_bass_kernel_spmd(nc, in_maps, list(range(N_CORES)))
    out = np.concatenate(
        [np.asarray(res.results[i]["out"]) for i in range(N_CORES)], axis=0
    )
    return out.astype(np.float32)
